# revision 5
# baseline (speedup 1.0000x reference)
"""ColorLoss (3D color histogram + L1) Trainium2 kernel, v3.

Strategy (data-parallel over batch, 8 cores):
  - Core i processes image i ([3,1024,1024]) plus 1/8 of the style image
    ([3,128,1024] row-slice).
  - Per pixel, channel bin indices r,g,b in [0,16) computed with ACT
    Relu-chains + float->int16 convert (round of t-0.5 == floor).
  - flat bin = key1 + 64*key2 with key1 = r + 16*(g&3), key2 = (g>>2) + 4*b.
  - 4096-bin joint histogram = 64x64 outer-product accumulation on the
    TensorEngine: PSUM[m,n] += sum_px E1[px,m] * E2[px,n]; two pixel blocks
    packed per matmul (M=N=128, diagonal blocks; off-diagonal discarded).
  - E1 (stationary, plane-major [128,64,2,T/2], contiguous per-plane dst):
    planes split DVE tensor_scalar(is_equal) at 4x bf16 rate (identity rows)
    and ACT Sign(k-j+0.5) (step rows, +-1, never 0 -> well-conditioned);
    a 64x64 inverse un-mixes on the host.
  - E2 (moving, pixel-major [128,T/2,2,64], contiguous per tau): single
    broadcast tensor_tensor(is_equal) against a static iota row, split by
    tau-range between GPSIMD and DVE (both write contiguous dst).
  - Host: tiny 64x64 un-mix per core, assemble histograms, L1 loss.
"""
import sys

sys.path.insert(0, "/opt/trn_rl_repo")
import os
import numpy as np
from contextlib import ExitStack

import ml_dtypes  # noqa: F401

# ---------------- tunables ----------------
T = 320            # pixels per partition per chunk (must be even)
N_E1_ACT = 34      # e1 planes generated on ScalarE (Sign step rows)
N_E1_DVE = 0       # e1 planes generated on DVE (rest go to GPSIMD)
GPS_FRAC = 0.0     # fraction of each chunk's taus encoded by GPSIMD (e2)
H, W = 1024, 1024
HW = H * W
IMG_PP = HW // 128          # pixels per partition for one image (8192)
STY_PP = 128 * W // 128     # pixels per partition for the style slice (1024)

_cache = {}


def _act_plane_set():
    """Choose e1 ACT planes (Sign(k-j+0.5) step rows); return set + M1inv."""
    k = np.arange(64)
    M1 = np.eye(64)
    act = sorted(set(np.round(np.linspace(1, 62, N_E1_ACT)).astype(int)))
    for j in act:
        M1[j] = np.sign(k - j + 0.5)
    cond = np.linalg.cond(M1)
    assert cond < 1e6, f"bad plane split, cond={cond}"
    return set(act), np.linalg.inv(M1)


def _build():
    import concourse.bacc as bacc
    import concourse.mybir as mybir
    from concourse.tile import TileContext

    F32 = mybir.dt.float32
    BF16 = mybir.dt.bfloat16
    I16 = mybir.dt.int16
    Alu = mybir.AluOpType
    Act = mybir.ActivationFunctionType

    act_set, M1inv = _act_plane_set()

    nc = bacc.Bacc("TRN2")
    img_d = nc.dram_tensor("img", [3, H, W], F32, kind="ExternalInput")
    sty_d = nc.dram_tensor("sty", [3, 128, W], F32, kind="ExternalInput")
    o_d = nc.dram_tensor("out", [2, 128, 128], F32, kind="ExternalOutput")

    img_v = [img_d[c, :, :].rearrange("(p r) w -> p (r w)", p=128) for c in range(3)]
    sty_v = [sty_d[c, :, :] for c in range(3)]

    def chunks(total):
        off, out = 0, []
        while off < total:
            tc_ = min(T, total - off)
            out.append((off, tc_))
            off += tc_
        return out

    img_chunks = chunks(IMG_PP)
    sty_chunks = chunks(STY_PP)

    with TileContext(nc) as tc:
        with ExitStack() as ctx:
            xpool = ctx.enter_context(tc.tile_pool(name="x", bufs=2))
            tpool = ctx.enter_context(tc.tile_pool(name="t", bufs=2))
            ipool = ctx.enter_context(tc.tile_pool(name="i", bufs=2))
            kpool = ctx.enter_context(tc.tile_pool(name="k", bufs=2))
            e1pool = ctx.enter_context(tc.tile_pool(name="e1", bufs=2))
            e2pool = ctx.enter_context(tc.tile_pool(name="e2", bufs=2))
            cpool = ctx.enter_context(tc.tile_pool(name="c", bufs=1))
            opool = ctx.enter_context(tc.tile_pool(name="o", bufs=1))
            pspool = ctx.enter_context(tc.tile_pool(name="ps", bufs=2, space="PSUM"))

            # constants
            iota = cpool.tile([128, 64], BF16, tag="iota")
            for j in range(64):
                nc.vector.memset(iota[:, j : j + 1], float(j))
            bcl1 = cpool.tile([128, 1], F32, tag="bcl1")
            nc.vector.memset(bcl1[:], 7.4)
            bcl2 = cpool.tile([128, 1], F32, tag="bcl2")
            nc.vector.memset(bcl2[:], 15.4)
            bias_j = cpool.tile([128, 64], F32, tag="biasj")
            for j in sorted(act_set):
                nc.vector.memset(bias_j[:, j : j + 1], 0.5 - float(j))

            ps_img = pspool.tile([128, 128], F32)
            ps_sty = pspool.tile([128, 128], F32)

            def do_chunk(views, off, tcw, ps, start, stop):
                th = tcw // 2
                s = int(round(GPS_FRAC * th))
                xt = xpool.tile([128, 3, T], F32, tag="xt")
                for c in range(3):
                    nc.sync.dma_start(xt[:, c, :tcw], views[c][:, off : off + tcw])
                ut = tpool.tile([128, 3, T], F32, tag="ut")
                tt = tpool.tile([128, 3, T], F32, tag="tt")
                for c in range(3):
                    nc.scalar.activation(ut[:, c, :tcw], xt[:, c, :tcw], Act.Relu,
                                         bias=bcl1[:], scale=-8.0)
                    nc.scalar.activation(tt[:, c, :tcw], ut[:, c, :tcw], Act.Relu,
                                         bias=bcl2[:], scale=-1.0)
                ii = ipool.tile([128, 4, T], I16, tag="ii")
                # floor via round(t - 0.5): rows r,g,b idx + gh
                for c in range(3):
                    nc.vector.tensor_scalar(ii[:, c, :tcw], tt[:, c, :tcw], 0.5, None,
                                            Alu.subtract)
                nc.vector.tensor_scalar(ii[:, 3, :tcw], tt[:, 1, :tcw], 0.25, 0.5,
                                        Alu.mult, Alu.subtract)
                # gl = g - 4*gh  (reuse row 1, stays i16)
                nc.vector.scalar_tensor_tensor(ii[:, 1, :tcw], ii[:, 3, :tcw], -4.0,
                                               ii[:, 1, :tcw], Alu.mult, Alu.add)
                kb = kpool.tile([128, 2, T], BF16, tag="kb")
                # key1 = 16*gl + r ; key2 = 4*b + gh  (i16 in -> bf16 out)
                nc.vector.scalar_tensor_tensor(kb[:, 0, :tcw], ii[:, 1, :tcw], 16.0,
                                               ii[:, 0, :tcw], Alu.mult, Alu.add)
                nc.vector.scalar_tensor_tensor(kb[:, 1, :tcw], ii[:, 2, :tcw], 4.0,
                                               ii[:, 3, :tcw], Alu.mult, Alu.add)
                key1 = kb[:, 0, :tcw]
                key2 = kb[:, 1, :tcw]

                # side1: plane-major [128, 64, 2, T//2]
                e1 = e1pool.tile([128, 64, 2, T // 2], BF16, tag="e1")
                k1v = key1.rearrange("p (b t) -> p b t", b=2)
                ndve = 0
                for j in range(64):
                    dst = e1[:, j, :, :th]
                    if j in act_set:
                        nc.scalar.activation(dst, k1v, Act.Sign,
                                             bias=bias_j[:, j : j + 1], scale=1.0)
                    elif ndve < N_E1_DVE:
                        nc.vector.tensor_scalar(dst, k1v, float(j), None, Alu.is_equal)
                        ndve += 1
                    else:
                        nc.gpsimd.tensor_scalar(dst, k1v, float(j), None, Alu.is_equal)

                # side2: pixel-major [128, T//2, 2, 64]; broadcast TT split
                # by tau range: GPSIMD takes [0, s), DVE takes [s, th)
                e2 = e2pool.tile([128, T // 2, 2, 64], BF16, tag="e2")
                k2v = key2.rearrange("p (b t) -> p t b", b=2)  # [128, th, 2]
                k2b = k2v.unsqueeze(3).broadcast_to([128, th, 2, 64])
                i2b = iota[:, :].unsqueeze(1).unsqueeze(2).broadcast_to([128, th, 2, 64])
                if s > 0:
                    nc.gpsimd.tensor_tensor(e2[:, :s, :, :], k2b[:, :s], i2b[:, :s],
                                            Alu.subtract)
                    nc.gpsimd.tensor_scalar(e2[:, :s, :, :], e2[:, :s, :, :], 0.0,
                                            None, Alu.is_equal)
                if s < th:
                    nc.vector.tensor_tensor(e2[:, s:th, :, :], k2b[:, s:], i2b[:, s:],
                                            Alu.is_equal)

                for tau in range(th):
                    nc.tensor.matmul(
                        ps[:],
                        e1[:, :, :, tau],
                        e2[:, tau, :, :],
                        start=(start and tau == 0),
                        stop=(stop and tau == th - 1),
                    )

            n_img = len(img_chunks)
            for ci, (off, tcw) in enumerate(img_chunks):
                do_chunk(img_v, off, tcw, ps_img, ci == 0, ci == n_img - 1)
            n_sty = len(sty_chunks)
            for ci, (off, tcw) in enumerate(sty_chunks):
                do_chunk(sty_v, off, tcw, ps_sty, ci == 0, ci == n_sty - 1)

            ostage = opool.tile([128, 2, 128], F32)
            nc.vector.tensor_copy(ostage[:, 0, :], ps_img[:])
            nc.vector.tensor_copy(ostage[:, 1, :], ps_sty[:])
            nc.sync.dma_start(o_d[0, :, :], ostage[:, 0, :])
            nc.sync.dma_start(o_d[1, :, :], ostage[:, 1, :])

    nc.finalize()
    return nc, M1inv


def _get_built():
    if "nc" not in _cache:
        nc, M1inv = _build()
        _cache["nc"] = nc
        _cache["M1inv"] = M1inv
    return _cache["nc"], _cache["M1inv"]


def _unmix(raw, M1inv):
    """raw [2,128,128] f32 -> (hist_img[4096], hist_sty[4096]) exact counts."""
    out = []
    for s in range(2):
        r = raw[s].astype(np.float64)
        # m = j1*2 + b ; n = b*64 + j2 ; diagonal blocks b==b'
        mixed = r[0::2, 0:64] + r[1::2, 64:128]   # [64 j1, 64 j2] = M1 @ H
        Hm = M1inv @ mixed
        out.append(np.rint(Hm))
    return out


def kernel(input, style_image, n_bins):
    assert int(n_bins) == 16
    from concourse import bass_utils

    nc, M1inv = _get_built()
    input = np.ascontiguousarray(np.asarray(input, dtype=np.float32))
    style = np.ascontiguousarray(np.asarray(style_image, dtype=np.float32))
    B = input.shape[0]
    assert B == 8 and input.shape == (8, 3, H, W)
    in_maps = [
        {
            "img": input[i],
            "sty": np.ascontiguousarray(style[0, :, 128 * i : 128 * (i + 1), :]),
        }
        for i in range(8)
    ]
    res = bass_utils.run_bass_kernel_spmd(nc, in_maps, core_ids=list(range(8)),
                                          **_cache.get("run_kwargs", {}))
    _cache["last_results"] = res
    hists = np.zeros((B, 4096), np.float64)
    sty_hist = np.zeros(4096, np.float64)
    for i in range(8):
        hi, hs = _unmix(res.results[i]["out"], M1inv)
        # flat = key1 + 64*key2 -> hist_flat[f] = H[j1=f%64, j2=f//64]
        hists[i] = hi.T.reshape(4096)
        sty_hist += hs.T.reshape(4096)
    cols = (hists / HW).astype(np.float32)
    target = (sty_hist / HW).astype(np.float32)
    loss = np.mean(np.abs(cols - target[None, :]).astype(np.float32))
    return np.float32(loss)


# revision 6
# speedup vs baseline: 6.1119x; 6.1119x over previous
"""ColorLoss (3D color histogram + L1) Trainium2 kernel, v3.

Strategy (data-parallel over batch, 8 cores):
  - Core i processes image i ([3,1024,1024]) plus 1/8 of the style image
    ([3,128,1024] row-slice).
  - Per pixel, channel bin indices r,g,b in [0,16) computed with ACT
    Relu-chains + float->int16 convert (round of t-0.5 == floor).
  - flat bin = key1 + 64*key2 with key1 = r + 16*(g&3), key2 = (g>>2) + 4*b.
  - 4096-bin joint histogram = 64x64 outer-product accumulation on the
    TensorEngine: PSUM[m,n] += sum_px E1[px,m] * E2[px,n]; two pixel blocks
    packed per matmul (M=N=128, diagonal blocks; off-diagonal discarded).
  - E1 (stationary, plane-major [128,64,2,T/2], contiguous per-plane dst):
    planes split DVE tensor_scalar(is_equal) at 4x bf16 rate (identity rows)
    and ACT Sign(k-j+0.5) (step rows, +-1, never 0 -> well-conditioned);
    a 64x64 inverse un-mixes on the host.
  - E2 (moving, pixel-major [128,T/2,2,64], contiguous per tau): single
    broadcast tensor_tensor(is_equal) against a static iota row, split by
    tau-range between GPSIMD and DVE (both write contiguous dst).
  - Host: tiny 64x64 un-mix per core, assemble histograms, L1 loss.
"""
import sys

sys.path.insert(0, "/opt/trn_rl_repo")
import os
import numpy as np
from contextlib import ExitStack

import ml_dtypes  # noqa: F401

# ---------------- tunables ----------------
T = 320            # pixels per partition per chunk (must be even)
N_E1_ACT = 41      # e1 planes generated on ScalarE (Sign step rows)
N_E1_DVE = 64      # e1 planes generated on DVE (rest go to GPSIMD)
GPS_FRAC = 0.0     # fraction of each chunk's taus encoded by GPSIMD (e2)
H, W = 1024, 1024
HW = H * W
IMG_PP = HW // 128          # pixels per partition for one image (8192)
STY_PP = 128 * W // 128     # pixels per partition for the style slice (1024)

_cache = {}


def _act_plane_set():
    """Choose e1 ACT planes (Sign(k-j+0.5) step rows); return set + M1inv."""
    k = np.arange(64)
    M1 = np.eye(64)
    act = sorted(set(np.round(np.linspace(1, 62, N_E1_ACT)).astype(int)))
    for j in act:
        M1[j] = np.sign(k - j + 0.5)
    cond = np.linalg.cond(M1)
    assert cond < 1e6, f"bad plane split, cond={cond}"
    return set(act), np.linalg.inv(M1)


def _build():
    import concourse.bacc as bacc
    import concourse.mybir as mybir
    from concourse.tile import TileContext

    F32 = mybir.dt.float32
    BF16 = mybir.dt.bfloat16
    I16 = mybir.dt.int16
    Alu = mybir.AluOpType
    Act = mybir.ActivationFunctionType

    act_set, M1inv = _act_plane_set()

    nc = bacc.Bacc("TRN2")
    img_d = nc.dram_tensor("img", [3, H, W], F32, kind="ExternalInput")
    sty_d = nc.dram_tensor("sty", [3, 128, W], F32, kind="ExternalInput")
    o_d = nc.dram_tensor("out", [2, 128, 128], F32, kind="ExternalOutput")

    img_v = [img_d[c, :, :].rearrange("(p r) w -> p (r w)", p=128) for c in range(3)]
    sty_v = [sty_d[c, :, :] for c in range(3)]

    def chunks(total):
        off, out = 0, []
        while off < total:
            tc_ = min(T, total - off)
            out.append((off, tc_))
            off += tc_
        return out

    img_chunks = chunks(IMG_PP)
    sty_chunks = chunks(STY_PP)

    with TileContext(nc) as tc:
        with ExitStack() as ctx:
            xpool = ctx.enter_context(tc.tile_pool(name="x", bufs=2))
            tpool = ctx.enter_context(tc.tile_pool(name="t", bufs=2))
            ipool = ctx.enter_context(tc.tile_pool(name="i", bufs=2))
            kpool = ctx.enter_context(tc.tile_pool(name="k", bufs=2))
            e1pool = ctx.enter_context(tc.tile_pool(name="e1", bufs=2))
            e2pool = ctx.enter_context(tc.tile_pool(name="e2", bufs=2))
            cpool = ctx.enter_context(tc.tile_pool(name="c", bufs=1))
            opool = ctx.enter_context(tc.tile_pool(name="o", bufs=1))
            pspool = ctx.enter_context(tc.tile_pool(name="ps", bufs=2, space="PSUM"))

            # constants
            iota = cpool.tile([128, 64], BF16, tag="iota")
            for j in range(64):
                nc.vector.memset(iota[:, j : j + 1], float(j))
            bcl1 = cpool.tile([128, 1], F32, tag="bcl1")
            nc.vector.memset(bcl1[:], 7.4)
            bcl2 = cpool.tile([128, 1], F32, tag="bcl2")
            nc.vector.memset(bcl2[:], 15.4)
            bias_j = cpool.tile([128, 64], F32, tag="biasj")
            for j in sorted(act_set):
                nc.vector.memset(bias_j[:, j : j + 1], 0.5 - float(j))

            ps_img = pspool.tile([128, 128], F32)
            ps_sty = pspool.tile([128, 128], F32)

            def do_chunk(views, off, tcw, ps, start, stop):
                th = tcw // 2
                s = int(round(GPS_FRAC * th))
                xt = xpool.tile([128, 3, T], F32, tag="xt")
                for c in range(3):
                    nc.sync.dma_start(xt[:, c, :tcw], views[c][:, off : off + tcw])
                ut = tpool.tile([128, 3, T], F32, tag="ut")
                tt = tpool.tile([128, 3, T], F32, tag="tt")
                for c in range(3):
                    nc.scalar.activation(ut[:, c, :tcw], xt[:, c, :tcw], Act.Relu,
                                         bias=bcl1[:], scale=-8.0)
                    nc.scalar.activation(tt[:, c, :tcw], ut[:, c, :tcw], Act.Relu,
                                         bias=bcl2[:], scale=-1.0)
                ii = ipool.tile([128, 4, T], I16, tag="ii")
                # floor via round(t - 0.5): rows r,g,b idx + gh
                for c in range(3):
                    nc.vector.tensor_scalar(ii[:, c, :tcw], tt[:, c, :tcw], 0.5, None,
                                            Alu.subtract)
                nc.vector.tensor_scalar(ii[:, 3, :tcw], tt[:, 1, :tcw], 0.25, 0.5,
                                        Alu.mult, Alu.subtract)
                # gl = g - 4*gh  (reuse row 1, stays i16)
                nc.vector.scalar_tensor_tensor(ii[:, 1, :tcw], ii[:, 3, :tcw], -4.0,
                                               ii[:, 1, :tcw], Alu.mult, Alu.add)
                kb = kpool.tile([128, 2, T], BF16, tag="kb")
                # key1 = 16*gl + r ; key2 = 4*b + gh  (i16 in -> bf16 out)
                nc.vector.scalar_tensor_tensor(kb[:, 0, :tcw], ii[:, 1, :tcw], 16.0,
                                               ii[:, 0, :tcw], Alu.mult, Alu.add)
                nc.vector.scalar_tensor_tensor(kb[:, 1, :tcw], ii[:, 2, :tcw], 4.0,
                                               ii[:, 3, :tcw], Alu.mult, Alu.add)
                key1 = kb[:, 0, :tcw]
                key2 = kb[:, 1, :tcw]

                # side1: plane-major [128, 64, 2, T//2]
                e1 = e1pool.tile([128, 64, 2, T // 2], BF16, tag="e1")
                k1v = key1.rearrange("p (b t) -> p b t", b=2)
                ndve = 0
                for j in range(64):
                    dst = e1[:, j, :, :th]
                    if j in act_set:
                        nc.scalar.activation(dst, k1v, Act.Sign,
                                             bias=bias_j[:, j : j + 1], scale=1.0)
                    elif ndve < N_E1_DVE:
                        nc.vector.tensor_scalar(dst, k1v, float(j), None, Alu.is_equal)
                        ndve += 1
                    else:
                        nc.gpsimd.tensor_scalar(dst, k1v, float(j), None, Alu.is_equal)

                # side2: pixel-major [128, T//2, 2, 64]; broadcast TT split
                # by tau range: GPSIMD takes [0, s), DVE takes [s, th)
                e2 = e2pool.tile([128, T // 2, 2, 64], BF16, tag="e2")
                k2v = key2.rearrange("p (b t) -> p t b", b=2)  # [128, th, 2]
                k2b = k2v.unsqueeze(3).broadcast_to([128, th, 2, 64])
                i2b = iota[:, :].unsqueeze(1).unsqueeze(2).broadcast_to([128, th, 2, 64])
                if s > 0:
                    nc.gpsimd.tensor_tensor(e2[:, :s, :, :], k2b[:, :s], i2b[:, :s],
                                            Alu.subtract)
                    nc.gpsimd.tensor_scalar(e2[:, :s, :, :], e2[:, :s, :, :], 0.0,
                                            None, Alu.is_equal)
                if s < th:
                    nc.vector.tensor_tensor(e2[:, s:th, :, :], k2b[:, s:], i2b[:, s:],
                                            Alu.is_equal)

                for tau in range(th):
                    nc.tensor.matmul(
                        ps[:],
                        e1[:, :, :, tau],
                        e2[:, tau, :, :],
                        start=(start and tau == 0),
                        stop=(stop and tau == th - 1),
                    )

            n_img = len(img_chunks)
            for ci, (off, tcw) in enumerate(img_chunks):
                do_chunk(img_v, off, tcw, ps_img, ci == 0, ci == n_img - 1)
            n_sty = len(sty_chunks)
            for ci, (off, tcw) in enumerate(sty_chunks):
                do_chunk(sty_v, off, tcw, ps_sty, ci == 0, ci == n_sty - 1)

            ostage = opool.tile([128, 2, 128], F32)
            nc.vector.tensor_copy(ostage[:, 0, :], ps_img[:])
            nc.vector.tensor_copy(ostage[:, 1, :], ps_sty[:])
            nc.sync.dma_start(o_d[0, :, :], ostage[:, 0, :])
            nc.sync.dma_start(o_d[1, :, :], ostage[:, 1, :])

    nc.finalize()
    return nc, M1inv


def _get_built():
    if "nc" not in _cache:
        nc, M1inv = _build()
        _cache["nc"] = nc
        _cache["M1inv"] = M1inv
    return _cache["nc"], _cache["M1inv"]


def _unmix(raw, M1inv):
    """raw [2,128,128] f32 -> (hist_img[4096], hist_sty[4096]) exact counts."""
    out = []
    for s in range(2):
        r = raw[s].astype(np.float64)
        # m = j1*2 + b ; n = b*64 + j2 ; diagonal blocks b==b'
        mixed = r[0::2, 0:64] + r[1::2, 64:128]   # [64 j1, 64 j2] = M1 @ H
        Hm = M1inv @ mixed
        out.append(np.rint(Hm))
    return out


def kernel(input, style_image, n_bins):
    assert int(n_bins) == 16
    from concourse import bass_utils

    nc, M1inv = _get_built()
    input = np.ascontiguousarray(np.asarray(input, dtype=np.float32))
    style = np.ascontiguousarray(np.asarray(style_image, dtype=np.float32))
    B = input.shape[0]
    assert B == 8 and input.shape == (8, 3, H, W)
    in_maps = [
        {
            "img": input[i],
            "sty": np.ascontiguousarray(style[0, :, 128 * i : 128 * (i + 1), :]),
        }
        for i in range(8)
    ]
    res = bass_utils.run_bass_kernel_spmd(nc, in_maps, core_ids=list(range(8)),
                                          **_cache.get("run_kwargs", {}))
    _cache["last_results"] = res
    hists = np.zeros((B, 4096), np.float64)
    sty_hist = np.zeros(4096, np.float64)
    for i in range(8):
        hi, hs = _unmix(res.results[i]["out"], M1inv)
        # flat = key1 + 64*key2 -> hist_flat[f] = H[j1=f%64, j2=f//64]
        hists[i] = hi.T.reshape(4096)
        sty_hist += hs.T.reshape(4096)
    cols = (hists / HW).astype(np.float32)
    target = (sty_hist / HW).astype(np.float32)
    loss = np.mean(np.abs(cols - target[None, :]).astype(np.float32))
    return np.float32(loss)


# revision 7
# speedup vs baseline: 6.2152x; 1.0169x over previous
"""ColorLoss (3D color histogram + L1) Trainium2 kernel, v3.

Strategy (data-parallel over batch, 8 cores):
  - Core i processes image i ([3,1024,1024]) plus 1/8 of the style image
    ([3,128,1024] row-slice).
  - Per pixel, channel bin indices r,g,b in [0,16) computed with ACT
    Relu-chains + float->int16 convert (round of t-0.5 == floor).
  - flat bin = key1 + 64*key2 with key1 = r + 16*(g&3), key2 = (g>>2) + 4*b.
  - 4096-bin joint histogram = 64x64 outer-product accumulation on the
    TensorEngine: PSUM[m,n] += sum_px E1[px,m] * E2[px,n]; two pixel blocks
    packed per matmul (M=N=128, diagonal blocks; off-diagonal discarded).
  - E1 (stationary, plane-major [128,64,2,T/2], contiguous per-plane dst):
    planes split DVE tensor_scalar(is_equal) at 4x bf16 rate (identity rows)
    and ACT Sign(k-j+0.5) (step rows, +-1, never 0 -> well-conditioned);
    a 64x64 inverse un-mixes on the host.
  - E2 (moving, pixel-major [128,T/2,2,64], contiguous per tau): single
    broadcast tensor_tensor(is_equal) against a static iota row, split by
    tau-range between GPSIMD and DVE (both write contiguous dst).
  - Host: tiny 64x64 un-mix per core, assemble histograms, L1 loss.
"""
import sys

sys.path.insert(0, "/opt/trn_rl_repo")
import os
import numpy as np
from contextlib import ExitStack

import ml_dtypes  # noqa: F401

# ---------------- tunables ----------------
T = 320            # pixels per partition per chunk (must be even)
N_E1_ACT = 44      # e1 planes generated on ScalarE (Sign step rows)
N_E1_DVE = 64      # e1 planes generated on DVE (rest go to GPSIMD)
GPS_FRAC = 0.0     # fraction of each chunk's taus encoded by GPSIMD (e2)
H, W = 1024, 1024
HW = H * W
IMG_PP = HW // 128          # pixels per partition for one image (8192)
STY_PP = 128 * W // 128     # pixels per partition for the style slice (1024)

_cache = {}


def _act_plane_set():
    """Choose e1 ACT planes (Sign(k-j+0.5) step rows); return set + M1inv."""
    k = np.arange(64)
    M1 = np.eye(64)
    act = sorted(set(np.round(np.linspace(1, 62, N_E1_ACT)).astype(int)))
    for j in act:
        M1[j] = np.sign(k - j + 0.5)
    cond = np.linalg.cond(M1)
    assert cond < 1e6, f"bad plane split, cond={cond}"
    return set(act), np.linalg.inv(M1)


def _build():
    import concourse.bacc as bacc
    import concourse.mybir as mybir
    from concourse.tile import TileContext

    F32 = mybir.dt.float32
    BF16 = mybir.dt.bfloat16
    I16 = mybir.dt.int16
    Alu = mybir.AluOpType
    Act = mybir.ActivationFunctionType

    act_set, M1inv = _act_plane_set()

    nc = bacc.Bacc("TRN2")
    img_d = nc.dram_tensor("img", [3, H, W], F32, kind="ExternalInput")
    sty_d = nc.dram_tensor("sty", [3, 128, W], F32, kind="ExternalInput")
    o_d = nc.dram_tensor("out", [2, 128, 128], F32, kind="ExternalOutput")

    img_v = [img_d[c, :, :].rearrange("(p r) w -> p (r w)", p=128) for c in range(3)]
    sty_v = [sty_d[c, :, :] for c in range(3)]

    def chunks(total):
        off, out = 0, []
        while off < total:
            tc_ = min(T, total - off)
            out.append((off, tc_))
            off += tc_
        return out

    img_chunks = chunks(IMG_PP)
    sty_chunks = chunks(STY_PP)

    with TileContext(nc) as tc:
        with ExitStack() as ctx:
            xpool = ctx.enter_context(tc.tile_pool(name="x", bufs=2))
            tpool = ctx.enter_context(tc.tile_pool(name="t", bufs=2))
            ipool = ctx.enter_context(tc.tile_pool(name="i", bufs=2))
            kpool = ctx.enter_context(tc.tile_pool(name="k", bufs=2))
            e1pool = ctx.enter_context(tc.tile_pool(name="e1", bufs=2))
            e2pool = ctx.enter_context(tc.tile_pool(name="e2", bufs=2))
            cpool = ctx.enter_context(tc.tile_pool(name="c", bufs=1))
            opool = ctx.enter_context(tc.tile_pool(name="o", bufs=1))
            pspool = ctx.enter_context(tc.tile_pool(name="ps", bufs=2, space="PSUM"))

            # constants
            iota = cpool.tile([128, 64], BF16, tag="iota")
            for j in range(64):
                nc.vector.memset(iota[:, j : j + 1], float(j))
            bcl1 = cpool.tile([128, 1], F32, tag="bcl1")
            nc.vector.memset(bcl1[:], 7.4)
            bcl2 = cpool.tile([128, 1], F32, tag="bcl2")
            nc.vector.memset(bcl2[:], 15.4)
            bias_j = cpool.tile([128, 64], F32, tag="biasj")
            for j in sorted(act_set):
                nc.vector.memset(bias_j[:, j : j + 1], 0.5 - float(j))

            ps_img = pspool.tile([128, 128], F32)
            ps_sty = pspool.tile([128, 128], F32)

            def do_chunk(views, off, tcw, ps, start, stop):
                th = tcw // 2
                s = int(round(GPS_FRAC * th))
                xt = xpool.tile([128, 3, T], F32, tag="xt")
                for c in range(3):
                    nc.sync.dma_start(xt[:, c, :tcw], views[c][:, off : off + tcw])
                ut = tpool.tile([128, 3, T], F32, tag="ut")
                tt = tpool.tile([128, 3, T], F32, tag="tt")
                for c in range(3):
                    nc.scalar.activation(ut[:, c, :tcw], xt[:, c, :tcw], Act.Relu,
                                         bias=bcl1[:], scale=-8.0)
                    nc.scalar.activation(tt[:, c, :tcw], ut[:, c, :tcw], Act.Relu,
                                         bias=bcl2[:], scale=-1.0)
                ii = ipool.tile([128, 4, T], I16, tag="ii")
                # floor via round(t - 0.5): rows r,g,b idx + gh
                for c in range(3):
                    nc.vector.tensor_scalar(ii[:, c, :tcw], tt[:, c, :tcw], 0.5, None,
                                            Alu.subtract)
                nc.vector.tensor_scalar(ii[:, 3, :tcw], tt[:, 1, :tcw], 0.25, 0.5,
                                        Alu.mult, Alu.subtract)
                # gl = g - 4*gh  (reuse row 1, stays i16)
                nc.vector.scalar_tensor_tensor(ii[:, 1, :tcw], ii[:, 3, :tcw], -4.0,
                                               ii[:, 1, :tcw], Alu.mult, Alu.add)
                kb = kpool.tile([128, 2, T], BF16, tag="kb")
                # key1 = 16*gl + r ; key2 = 4*b + gh  (i16 in -> bf16 out)
                nc.vector.scalar_tensor_tensor(kb[:, 0, :tcw], ii[:, 1, :tcw], 16.0,
                                               ii[:, 0, :tcw], Alu.mult, Alu.add)
                nc.vector.scalar_tensor_tensor(kb[:, 1, :tcw], ii[:, 2, :tcw], 4.0,
                                               ii[:, 3, :tcw], Alu.mult, Alu.add)
                key1 = kb[:, 0, :tcw]
                key2 = kb[:, 1, :tcw]

                # side1: plane-major [128, 64, 2, T//2]
                e1 = e1pool.tile([128, 64, 2, T // 2], BF16, tag="e1")
                k1v = key1.rearrange("p (b t) -> p b t", b=2)
                ndve = 0
                for j in range(64):
                    dst = e1[:, j, :, :th]
                    if j in act_set:
                        nc.scalar.activation(dst, k1v, Act.Sign,
                                             bias=bias_j[:, j : j + 1], scale=1.0)
                    elif ndve < N_E1_DVE:
                        nc.vector.tensor_scalar(dst, k1v, float(j), None, Alu.is_equal)
                        ndve += 1
                    else:
                        nc.gpsimd.tensor_scalar(dst, k1v, float(j), None, Alu.is_equal)

                # side2: pixel-major [128, T//2, 2, 64]; broadcast TT split
                # by tau range: GPSIMD takes [0, s), DVE takes [s, th)
                e2 = e2pool.tile([128, T // 2, 2, 64], BF16, tag="e2")
                k2v = key2.rearrange("p (b t) -> p t b", b=2)  # [128, th, 2]
                k2b = k2v.unsqueeze(3).broadcast_to([128, th, 2, 64])
                i2b = iota[:, :].unsqueeze(1).unsqueeze(2).broadcast_to([128, th, 2, 64])
                if s > 0:
                    nc.gpsimd.tensor_tensor(e2[:, :s, :, :], k2b[:, :s], i2b[:, :s],
                                            Alu.subtract)
                    nc.gpsimd.tensor_scalar(e2[:, :s, :, :], e2[:, :s, :, :], 0.0,
                                            None, Alu.is_equal)
                if s < th:
                    nc.vector.tensor_tensor(e2[:, s:th, :, :], k2b[:, s:], i2b[:, s:],
                                            Alu.is_equal)

                for tau in range(th):
                    nc.tensor.matmul(
                        ps[:],
                        e1[:, :, :, tau],
                        e2[:, tau, :, :],
                        start=(start and tau == 0),
                        stop=(stop and tau == th - 1),
                    )

            n_img = len(img_chunks)
            for ci, (off, tcw) in enumerate(img_chunks):
                do_chunk(img_v, off, tcw, ps_img, ci == 0, ci == n_img - 1)
            n_sty = len(sty_chunks)
            for ci, (off, tcw) in enumerate(sty_chunks):
                do_chunk(sty_v, off, tcw, ps_sty, ci == 0, ci == n_sty - 1)

            ostage = opool.tile([128, 2, 128], F32)
            nc.vector.tensor_copy(ostage[:, 0, :], ps_img[:])
            nc.vector.tensor_copy(ostage[:, 1, :], ps_sty[:])
            nc.sync.dma_start(o_d[0, :, :], ostage[:, 0, :])
            nc.sync.dma_start(o_d[1, :, :], ostage[:, 1, :])

    nc.finalize()
    return nc, M1inv


def _get_built():
    if "nc" not in _cache:
        nc, M1inv = _build()
        _cache["nc"] = nc
        _cache["M1inv"] = M1inv
    return _cache["nc"], _cache["M1inv"]


def _unmix(raw, M1inv):
    """raw [2,128,128] f32 -> (hist_img[4096], hist_sty[4096]) exact counts."""
    out = []
    for s in range(2):
        r = raw[s].astype(np.float64)
        # m = j1*2 + b ; n = b*64 + j2 ; diagonal blocks b==b'
        mixed = r[0::2, 0:64] + r[1::2, 64:128]   # [64 j1, 64 j2] = M1 @ H
        Hm = M1inv @ mixed
        out.append(np.rint(Hm))
    return out


def kernel(input, style_image, n_bins):
    assert int(n_bins) == 16
    from concourse import bass_utils

    nc, M1inv = _get_built()
    input = np.ascontiguousarray(np.asarray(input, dtype=np.float32))
    style = np.ascontiguousarray(np.asarray(style_image, dtype=np.float32))
    B = input.shape[0]
    assert B == 8 and input.shape == (8, 3, H, W)
    in_maps = [
        {
            "img": input[i],
            "sty": np.ascontiguousarray(style[0, :, 128 * i : 128 * (i + 1), :]),
        }
        for i in range(8)
    ]
    res = bass_utils.run_bass_kernel_spmd(nc, in_maps, core_ids=list(range(8)),
                                          **_cache.get("run_kwargs", {}))
    _cache["last_results"] = res
    hists = np.zeros((B, 4096), np.float64)
    sty_hist = np.zeros(4096, np.float64)
    for i in range(8):
        hi, hs = _unmix(res.results[i]["out"], M1inv)
        # flat = key1 + 64*key2 -> hist_flat[f] = H[j1=f%64, j2=f//64]
        hists[i] = hi.T.reshape(4096)
        sty_hist += hs.T.reshape(4096)
    cols = (hists / HW).astype(np.float32)
    target = (sty_hist / HW).astype(np.float32)
    loss = np.mean(np.abs(cols - target[None, :]).astype(np.float32))
    return np.float32(loss)
